# revision 1
# baseline (speedup 1.0000x reference)
"""Trainium2 Bass kernel for nn_Columbina_Model (2-layer hetero GAT).

Strategy: dst-shard gene nodes across 8 cores (12500 each, padded to 12544).
Host sorts each core's edges by (dst tile, src region), groups them into
128-edge chunks per (tile, region). On device: encoders + BN stats
(AllReduce) per-shard; BN folded into slim table rows
[g post-BN (32 bf16) | a_s (4 bf16) | pad] of 256B; tables AllGathered.
Edge phase gathers rows via batched SWDGE dma_gather (one instruction per
run of up to 8 chunks sharing a src region — int16 region-local indices),
computes attention (leaky-relu + exp), and segment-sums per-head
exp-weighted g into per-node-tile PSUM accumulators with one-hot matmuls.
Per-tile epilogue normalizes per head and reconstructs the H*C GAT output
with a single stacked-W matmul, then adds bias + residual.
"""
import json
import sys
import types

import numpy as np
import ml_dtypes

H, C = 4, 32
NG = ND = 100000
M = 8
RN = 12500
RPAD = 12544            # 98 * 128
NTILES = RPAD // 128
NT = RPAD * M           # 100352 table rows
DUMMY = RN              # dummy table row (core-0 own pad; a_s=-1e9)
ROWW = 36               # table row elements (bf16) = 72B
GB = 8                  # chunks per compute group
BN_EPS = 1e-5

bf16 = ml_dtypes.bfloat16

LAST_RESULT = None

# --------------------------------------------------------------------------
# runtime patches (this container lacks antenv.axon_hooks; walrus rejects >1
# sync wait per instruction)
# --------------------------------------------------------------------------


def _install_patches():
    if "antenv.axon_hooks" not in sys.modules:
        try:
            import antenv
            from trn_agent_boot.trn_boot import _ntff_profile_via_ctypes

            m = types.ModuleType("antenv.axon_hooks")
            _state = {"hook": _ntff_profile_via_ctypes("/opt/axon/libaxon_pjrt.so")}
            m.get_axon_ntff_profile_hook = lambda: _state["hook"]
            m.set_axon_ntff_profile_hook = lambda h: _state.__setitem__("hook", h)
            sys.modules["antenv.axon_hooks"] = m
            antenv.axon_hooks = m
        except Exception:
            pass

    import concourse.tile as tile
    from concourse.vector_clock import ScopedClock

    if not getattr(tile.TileContext, "_drain_patched", False):
        def _drain_and_barrier(self, tick_clock, wait_clock):
            gc = tick_clock.global_clock
            for proc in range(len(gc)):
                tick = gc[proc]
                if tick > 0:
                    partial = ScopedClock()
                    partial.require_at_least(None, proc, tick)
                    nop = self.nc.sync.nop(nofuse=True)
                    wait_clock.add_sem_waits(nop.ins, partial)
            self.nc.sync.drain()
            self.nc.all_engine_barrier()
            assert self.sems is not None
            popped = self.nc._tile_sem_poison_stack.pop()
            assert popped is self._sem_poison
            self.nc.clear_and_free_semaphores(list(self.sems.allocated().values()))
            self.nc.all_engine_barrier()

        tile.TileContext._drain_and_barrier = _drain_and_barrier
        tile.TileContext._drain_patched = True

    import concourse.bass_utils as bu
    import concourse.bass2jax as b2j

    if not getattr(bu, "_compile_patched", False):
        def _split(bir_json):
            d = json.loads(bir_json)
            for f in d.get("functions", []):
                for bb in f.get("blocks", []):
                    new_insts = []
                    for inst in bb["instructions"]:
                        si = inst.get("sync_info")
                        ow = (si or {}).get("on_wait") or []
                        if len(ow) > 1:
                            for i, w in enumerate(ow[:-1]):
                                new_insts.append({
                                    "debug": inst.get("debug", 0),
                                    "engine": inst["engine"],
                                    "ins": [], "outs": [],
                                    "name": f"{inst['name']}-w{i}",
                                    "opcode": "NoOp",
                                    "sync_info": {"on_update": [],
                                                  "on_wait": [w]},
                                })
                            si["on_wait"] = ow[-1:]
                        new_insts.append(inst)
                    bb["instructions"] = new_insts
            return json.dumps(d).encode()

        orig = bu.compile_bir_kernel

        def compile_bir_kernel(bir_json, tmpdir, neff_name="file.neff"):
            return orig(_split(bir_json), tmpdir, neff_name)

        bu.compile_bir_kernel = compile_bir_kernel
        b2j.compile_bir_kernel = compile_bir_kernel
        bu._compile_patched = True


# --------------------------------------------------------------------------
# host-side edge preprocessing
# --------------------------------------------------------------------------


def _fold_att(W, att):
    # W [32, 128], att [H, C] -> [32, H]
    return np.einsum('jhc,hc->jh', np.asarray(W).reshape(32, H, C), np.asarray(att))


def _remap(v):
    return (v // RN) * RPAD + (v % RN)


class _Sched:
    pass


def _prep_edges(src, dst):
    """Chunk schedule shared by all cores + per-core idx/seg planes."""
    src = _remap(np.asarray(src).astype(np.int64))
    dst = np.asarray(dst).astype(np.int64)

    percore = []
    counts = np.zeros((M, NTILES), np.int64)
    for k in range(M):
        m = (dst >= k * RN) & (dst < (k + 1) * RN)
        d = dst[m] - k * RN
        t = d >> 7
        order = np.lexsort((src[m], t))  # by (tile, src idx)
        percore.append((src[m][order], (d & 127)[order], t[order]))
        counts[k] = np.bincount(t, minlength=NTILES)

    cpt = np.maximum(1, (counts.max(axis=0) + 127) // 128)
    nch = int(cpt.sum())
    chunk_base = np.concatenate([[0], np.cumsum(cpt)])[:-1]
    tiles = np.repeat(np.arange(NTILES), cpt)

    starts = np.zeros(nch, bool)
    stops = np.zeros(nch, bool)
    starts[0] = True
    starts[1:] = tiles[1:] != tiles[:-1]
    stops[-1] = True
    stops[:-1] = tiles[1:] != tiles[:-1]

    idx32 = np.full((M, nch, 128), DUMMY, np.int32)
    seg = np.zeros((M, nch, 128), np.float32)
    for k in range(M):
        sk, dk, tk = percore[k]
        tile_start = np.concatenate([[0], np.cumsum(counts[k])])[:-1]
        pos = np.arange(len(sk)) - tile_start[tk]
        gchunk = chunk_base[tk] + (pos >> 7)
        slot = pos & 127
        idx32[k, gchunk, slot] = sk
        seg[k, gchunk, slot] = dk

    s = _Sched()
    s.nch = nch
    s.tiles, s.starts, s.stops = tiles, starts, stops
    s.planes = np.ascontiguousarray(np.transpose(idx32, (0, 2, 1)))
    s.segs = np.ascontiguousarray(np.transpose(seg, (0, 2, 1))).astype(bf16)
    return s


def _wstack(W):
    # W [32, H*C] -> [H*C, 32]: out[h*32+j, c] = W[j, h*32+c] / H
    Wr = np.asarray(W, np.float32).reshape(32, H, C)
    return (np.transpose(Wr, (1, 0, 2)).reshape(H * C, C) / H)


# --------------------------------------------------------------------------
# device kernel builder
# --------------------------------------------------------------------------


def _build(s1, s2):
    import concourse.bass as bass
    import concourse.mybir as mybir
    import concourse.tile as tile
    from concourse import bacc
    from concourse.bass import AP, IndirectOffsetOnAxis

    dt = mybir.dt
    nc = bacc.Bacc()

    def par(name, shape, dty, out=False):
        return nc.declare_dram_parameter(name, shape, dty, isOutput=out)

    xg = par("xg", [128, RPAD], dt.bfloat16)        # transposed gene shard
    xd = par("xd", [96, RPAD], dt.bfloat16)         # transposed drug shard
    wg = par("wg", [128, 32], dt.bfloat16)
    wd = par("wd", [96, 32], dt.bfloat16)
    bgv = par("bgv", [32, 1], dt.float32)           # encoder biases
    bdv = par("bdv", [32, 1], dt.float32)
    statc = par("statc", [32, 4], dt.float32)       # pad-col stat corrections
    gamb = par("gamb", [32, 4], dt.float32)         # gamma/beta g, d
    w1c = par("w1c", [32, 40], dt.float32)          # [I | A1s | A1d]
    w2c = par("w2c", [32, 36], dt.float32)          # [I | A2s]
    a2d = par("a2d", [32, 4], dt.bfloat16)          # fold(W2, att_d2)
    wst1 = par("wst1", [128, 32], dt.bfloat16)      # stacked W1 / H
    wst2 = par("wst2", [128, 32], dt.bfloat16)      # stacked W2 / H
    bias1r = par("bias1r", [128, 32], dt.float32)
    bias2r = par("bias2r", [128, 32], dt.float32)
    iotap = par("iotap", [128, 128], dt.bfloat16)
    i1 = par("i1", [128, s1.nch], dt.int32)
    g1s = par("g1s", [128, s1.nch], dt.bfloat16)
    i2 = par("i2", [128, s2.nch], dt.int32)
    g2s = par("g2s", [128, s2.nch], dt.bfloat16)
    outp = par("out", [RPAD, 32], dt.float32, out=True)

    with tile.TileContext(nc) as tc:
        with (
            tc.tile_pool(name="const", bufs=1) as cp,
            tc.tile_pool(name="enc", bufs=2) as ep,
            tc.tile_pool(name="work", bufs=4) as wp,
            tc.tile_pool(name="dram", bufs=1, space="DRAM") as dp,
        ):
            from concourse.masks import make_identity
            # ---------------- constants / params to SBUF ----------------
            t_wg = cp.tile([128, 32], dt.bfloat16)
            t_wd = cp.tile([96, 32], dt.bfloat16)
            t_bg = cp.tile([32, 1], dt.float32)
            t_bd = cp.tile([32, 1], dt.float32)
            t_statc = cp.tile([32, 4], dt.float32)
            t_gamb = cp.tile([32, 4], dt.float32)
            t_w1c = cp.tile([32, 40], dt.float32)
            t_w2c = cp.tile([32, 36], dt.float32)
            t_a2d = cp.tile([32, 4], dt.bfloat16)
            t_wst1 = cp.tile([128, 32], dt.bfloat16)
            t_wst2 = cp.tile([128, 32], dt.bfloat16)
            t_b1 = cp.tile([128, 32], dt.float32)
            t_b2 = cp.tile([128, 32], dt.float32)
            t_iota = cp.tile([128, 128], dt.bfloat16)
            t_i1 = cp.tile([128, s1.nch], dt.int32)
            t_s1g = cp.tile([128, s1.nch], dt.bfloat16)
            t_i2 = cp.tile([128, s2.nch], dt.int32)
            t_s2g = cp.tile([128, s2.nch], dt.bfloat16)
            ident = cp.tile([128, 128], dt.bfloat16)
            make_identity(nc, ident[:])
            for t, p in [(t_wg, wg), (t_wd, wd), (t_bg, bgv), (t_bd, bdv),
                         (t_statc, statc), (t_gamb, gamb), (t_w1c, w1c),
                         (t_w2c, w2c), (t_a2d, a2d), (t_wst1, wst1),
                         (t_wst2, wst2), (t_b1, bias1r), (t_b2, bias2r),
                         (t_iota, iotap), (t_i1, i1), (t_s1g, g1s),
                         (t_i2, i2), (t_s2g, g2s)]:
                nc.sync.dma_start(out=t[:], in_=p[:])


            # DRAM internals
            table1 = dp.tile([NT, ROWW], dt.bfloat16)
            table2 = dp.tile([NT, ROWW], dt.bfloat16)
            own1 = dp.tile([RPAD, ROWW], dt.bfloat16)
            own2 = dp.tile([RPAD, ROWW], dt.bfloat16)
            st_in = dp.tile([32, 4], dt.float32)
            st_out = dp.tile([32, 4], dt.float32)

            yp = tc.tile_pool(name="yp", bufs=1)
            ypool = yp.__enter__()
            enc_psum = tc.tile_pool(name="ps_enc", bufs=2, space="PSUM")
            pp = enc_psum.__enter__()

            # ---------------- encoders: y_T = relu(W^T x_T + b) ---------
            def encode(xT, wT, bT, npart, tag):
                yT = ypool.tile([33, RPAD], dt.bfloat16, tag=f"y{tag}")
                st = ep.tile([32, 50], dt.float32, tag=f"st{tag}")
                for i in range(25):
                    lo = i * 512
                    w = min(512, RPAD - lo)
                    x512 = ep.tile([128, 512], dt.bfloat16, tag=f"x{tag}",
                                   bufs=3)
                    nc.sync.dma_start(out=x512[:npart, :w],
                                      in_=xT[:, lo:lo + w])
                    ps = pp.tile([32, 512], dt.float32, space="PSUM", tag="encp")
                    nc.tensor.matmul(ps[:, :w], lhsT=wT[:npart, :],
                                     rhs=x512[:npart, :w],
                                     start=True, stop=True)
                    nc.scalar.activation(
                        yT[0:32, lo:lo + w], ps[:, :w],
                        mybir.ActivationFunctionType.Relu, bias=bT[:],
                        accum_out=st[:, i:i + 1])
                    sq = ep.tile([32, 512], dt.float32, tag=f"sq{tag}")
                    nc.scalar.activation(
                        sq[:, :w], yT[0:32, lo:lo + w],
                        mybir.ActivationFunctionType.Square,
                        accum_out=st[:, 25 + i:26 + i])
                nc.vector.memset(yT[32:33, :], 1.0)
                s1t = ep.tile([32, 2], dt.float32, tag=f"s1{tag}")
                nc.vector.tensor_reduce(s1t[:, 0:1], st[:, 0:25],
                                        mybir.AxisListType.X, mybir.AluOpType.add)
                nc.vector.tensor_reduce(s1t[:, 1:2], st[:, 25:50],
                                        mybir.AxisListType.X, mybir.AluOpType.add)
                return yT, s1t

            ygT, sg = encode(xg, t_wg, t_bg, 128, "g")
            ydT, sd = encode(xd, t_wd, t_bd, 96, "d")

            # stats allreduce: [32,4] = [sum_g, sq_g, sum_d, sq_d] - corr
            packed = ep.tile([32, 4], dt.float32, tag="pk")
            nc.vector.tensor_copy(out=packed[:, 0:2], in_=sg[:])
            nc.vector.tensor_copy(out=packed[:, 2:4], in_=sd[:])
            nc.vector.tensor_sub(out=packed[:], in0=packed[:], in1=t_statc[:])
            nc.sync.dma_start(out=st_in[:], in_=packed[:])
            nc.gpsimd.collective_compute(
                "AllReduce", mybir.AluOpType.add,
                replica_groups=[list(range(M))],
                ins=[st_in.opt()], outs=[st_out.opt()])
            tstat = ep.tile([32, 4], dt.float32, tag="ts")
            nc.gpsimd.dma_start(out=tstat[:], in_=st_out[:])

            # BN fold: s = gamma/sqrt(var+eps), t = beta - mu*s  (cols: g, d)
            sfold = ep.tile([32, 2], dt.float32, tag="sf")
            tfold = ep.tile([32, 2], dt.float32, tag="tf")
            mu = ep.tile([32, 2], dt.float32, tag="mu")
            var = ep.tile([32, 2], dt.float32, tag="va")
            tmp = ep.tile([32, 2], dt.float32, tag="tm")
            nc.vector.tensor_scalar(out=mu[:, 0:1], in0=tstat[:, 0:1],
                                    scalar1=1.0 / NG, scalar2=None,
                                    op0=mybir.AluOpType.mult)
            nc.vector.tensor_scalar(out=mu[:, 1:2], in0=tstat[:, 2:3],
                                    scalar1=1.0 / ND, scalar2=None,
                                    op0=mybir.AluOpType.mult)
            nc.vector.tensor_scalar(out=var[:, 0:1], in0=tstat[:, 1:2],
                                    scalar1=1.0 / NG, scalar2=None,
                                    op0=mybir.AluOpType.mult)
            nc.vector.tensor_scalar(out=var[:, 1:2], in0=tstat[:, 3:4],
                                    scalar1=1.0 / ND, scalar2=None,
                                    op0=mybir.AluOpType.mult)
            nc.vector.tensor_mul(out=tmp[:], in0=mu[:], in1=mu[:])
            nc.vector.tensor_sub(out=var[:], in0=var[:], in1=tmp[:])
            nc.vector.tensor_scalar(out=var[:], in0=var[:], scalar1=BN_EPS,
                                    scalar2=None, op0=mybir.AluOpType.add)
            nc.scalar.activation(tmp[:], var[:], mybir.ActivationFunctionType.Sqrt)
            nc.vector.reciprocal(out=tmp[:], in_=tmp[:])
            nc.vector.tensor_mul(out=sfold[:, 0:1], in0=t_gamb[:, 0:1],
                                 in1=tmp[:, 0:1])
            nc.vector.tensor_mul(out=sfold[:, 1:2], in0=t_gamb[:, 2:3],
                                 in1=tmp[:, 1:2])
            nc.vector.tensor_mul(out=tfold[:], in0=mu[:], in1=sfold[:])
            nc.vector.tensor_sub(out=tfold[:, 0:1], in0=t_gamb[:, 1:2],
                                 in1=tfold[:, 0:1])
            nc.vector.tensor_sub(out=tfold[:, 1:2], in0=t_gamb[:, 3:4],
                                 in1=tfold[:, 1:2])

            # waug [33, n]: rows 0..32 = diag-scaled [I|A..], row 32 = t@[..]
            def build_waug(wcomb, ncols, sf_col, tf_col, tag):
                wa = cp.tile([33, ncols], dt.bfloat16, tag=f"wa{tag}")
                scaled = ep.tile([32, ncols], dt.float32, tag=f"sc{tag}")
                nc.vector.tensor_tensor(
                    out=scaled[:], in0=wcomb[:, :ncols],
                    in1=sf_col.to_broadcast([32, ncols]),
                    op=mybir.AluOpType.mult)
                nc.vector.tensor_copy(out=wa[0:32, :], in_=scaled[:])
                tr = pp.tile([1, 40], dt.float32, space="PSUM", tag="warow")
                tscaled = ep.tile([32, 1], dt.float32, tag=f"tsc{tag}")
                nc.vector.tensor_copy(out=tscaled[:], in_=tf_col)
                nc.tensor.matmul(tr[:, :ncols], lhsT=tscaled[:],
                                 rhs=wcomb[:, :ncols], start=True, stop=True)
                nc.vector.tensor_copy(out=wa[32:33, :], in_=tr[:, :ncols])
                return wa

            waug1 = build_waug(t_w1c, 40, sfold[:, 0:1], tfold[:, 0:1], "1")
            waug2 = build_waug(t_w2c, 36, sfold[:, 1:2], tfold[:, 1:2], "2")

            # ---------------- own table builds ----------------
            a_d1 = cp.tile([128, NTILES * 4], dt.bfloat16)
            a_d2 = cp.tile([128, NTILES * 4], dt.bfloat16)
            a_s1o = cp.tile([128, NTILES * 4], dt.bfloat16)
            a_s2o = cp.tile([128, NTILES * 4], dt.bfloat16)
            g_own = cp.tile([128, NTILES * 32], dt.float32)
            d_own = cp.tile([128, NTILES * 32], dt.float32)
            g1_own = cp.tile([128, NTILES * 32], dt.float32)

            for j in range(NTILES):
                ps = pp.tile([128, 40], dt.float32, space="PSUM", tag="tb")
                nc.tensor.matmul(ps[:], lhsT=ygT[:, bass.ts(j, 128)],
                                 rhs=waug1[:], start=True, stop=True)
                row = wp.tile([128, ROWW], dt.bfloat16, tag="tbr")
                nc.vector.tensor_copy(out=row[:, 0:36], in_=ps[:, 0:36])
                nc.sync.dma_start(out=own1[bass.ts(j, 128), :], in_=row[:])
                nc.vector.tensor_copy(out=a_d1[:, bass.ts(j, 4)],
                                      in_=ps[:, 36:40])
                nc.vector.tensor_copy(out=a_s1o[:, bass.ts(j, 4)],
                                      in_=ps[:, 32:36])
                nc.vector.tensor_copy(out=g_own[:, bass.ts(j, 32)],
                                      in_=ps[:, 0:32])
            for j in range(NTILES):
                ps = pp.tile([128, 36], dt.float32, space="PSUM", tag="tb2")
                nc.tensor.matmul(ps[:], lhsT=ydT[:, bass.ts(j, 128)],
                                 rhs=waug2[:], start=True, stop=True)
                row = wp.tile([128, ROWW], dt.bfloat16, tag="tbr2")
                nc.vector.tensor_copy(out=row[:, 0:36], in_=ps[:])
                nc.sync.dma_start(out=own2[bass.ts(j, 128), :], in_=row[:])
                nc.vector.tensor_copy(out=a_s2o[:, bass.ts(j, 4)],
                                      in_=ps[:, 32:36])
                nc.vector.tensor_copy(out=d_own[:, bass.ts(j, 32)],
                                      in_=ps[:, 0:32])

            # dummy rows in the own-shard pad region (a_s = -1e9 -> exp = 0)
            drow = wp.tile([8, ROWW], dt.bfloat16, tag="drow")
            nc.vector.memset(drow[:], 0.0)
            nc.vector.memset(drow[:, 32:36], -1e9)
            nc.sync.dma_start(out=own1[RN:RN + 8, :], in_=drow[:])
            nc.sync.dma_start(out=own2[RN:RN + 8, :], in_=drow[:])

            yp.__exit__(None, None, None)
            enc_psum.__exit__(None, None, None)
            edge_psum = tc.tile_pool(name="ps_edge", bufs=2, space="PSUM")
            pp = edge_psum.__enter__()
            # AllGather tables
            nc.gpsimd.collective_compute(
                "AllGather", mybir.AluOpType.bypass,
                replica_groups=[list(range(M))],
                ins=[own1.opt()], outs=[table1[:]])
            nc.gpsimd.collective_compute(
                "AllGather", mybir.AluOpType.bypass,
                replica_groups=[list(range(M))],
                ins=[own2.opt()], outs=[table2[:]])

            # ---------------- edge phase ----------------
            def apv(t, part, dims, offset=0):
                a = t[:]
                return AP(a.tensor, a.offset + offset,
                          [(a.ap[0][0], part)] + list(dims))

            def edge_phase(sched, table, t_idx, t_sg, a_d):
                """Batched edge pipeline; yields (tile, psum_acc) per dst
                node tile."""
                live = {}
                for c0 in range(0, sched.nch, GB):
                    G = min(GB, sched.nch - c0)
                    g4 = wp.tile([128, GB, ROWW], dt.bfloat16, tag="eg",
                                 bufs=3)
                    for i in range(G):
                        nc.gpsimd.indirect_dma_start(
                            out=g4[:, i, :], out_offset=None, in_=table[:],
                            in_offset=IndirectOffsetOnAxis(
                                ap=t_idx[:, c0 + i:c0 + i + 1], axis=0))
                    P4 = wp.tile([128, GB, 128], dt.bfloat16, tag="eP", bufs=2)
                    pst = t_sg[:].ap[0][0]
                    nc.vector.tensor_tensor(
                        out=P4[:, :G, :],
                        in0=AP(t_sg[:].tensor, t_sg[:].offset + c0,
                               [(pst, 128), (1, G), (0, 128)]),
                        in1=apv(t_iota, 128, [(0, G), (1, 128)]),
                        op=mybir.AluOpType.is_equal)
                    ptp4 = pp.tile([128, GB, 128], dt.bfloat16, space="PSUM",
                                   tag="ptp", bufs=1)
                    for i in range(G):
                        nc.tensor.transpose(out=ptp4[:, i, :], in_=P4[:, i, :],
                                            identity=ident[:])
                    PT4 = wp.tile([128, GB, 128], dt.bfloat16, tag="ePT",
                                  bufs=2)
                    nc.scalar.activation(
                        apv(PT4, 128, [(1, G * 128)]),
                        apv(ptp4, 128, [(1, G * 128)]),
                        mybir.ActivationFunctionType.Copy)
                    pa4 = pp.tile([128, GB, 4], dt.float32, space="PSUM",
                                  tag="pa", bufs=1)
                    for i in range(G):
                        nc.tensor.matmul(
                            pa4[:, i, :], lhsT=PT4[:, i, :],
                            rhs=a_d[:, bass.ts(int(sched.tiles[c0 + i]), 4)],
                            start=True, stop=True)
                    alpha4 = wp.tile([128, GB, 4], dt.float32, tag="eal",
                                     bufs=2)
                    nc.vector.tensor_tensor(
                        out=alpha4[:, :G, :],
                        in0=apv(g4, 128, [(ROWW, G), (1, 4)], offset=32),
                        in1=apv(pa4, 128, [(4, G), (1, 4)]),
                        op=mybir.AluOpType.add)
                    lr4 = wp.tile([128, GB, 4], dt.float32, tag="elr", bufs=2)
                    nc.vector.scalar_tensor_tensor(
                        out=apv(lr4, 128, [(1, G * 4)]),
                        in0=apv(alpha4, 128, [(1, G * 4)]), scalar=0.2,
                        in1=apv(alpha4, 128, [(1, G * 4)]),
                        op0=mybir.AluOpType.mult, op1=mybir.AluOpType.max)
                    rhs4 = wp.tile([128, GB, 132], dt.bfloat16, tag="erh",
                                   bufs=2)
                    nc.scalar.activation(
                        apv(rhs4, 128, [(132, G), (1, 4)], offset=128),
                        apv(lr4, 128, [(4, G), (1, 4)]),
                        mybir.ActivationFunctionType.Exp)
                    nc.vector.tensor_tensor(
                        out=apv(rhs4, 128, [(132, G), (32, 4), (1, 32)]),
                        in0=apv(g4, 128, [(ROWW, G), (0, 4), (1, 32)]),
                        in1=apv(rhs4, 128, [(132, G), (1, 4), (0, 32)],
                                offset=128),
                        op=mybir.AluOpType.mult)
                    for i in range(G):
                        c = c0 + i
                        t = int(sched.tiles[c])
                        if sched.starts[c]:
                            live[t] = pp.tile([128, 132], dt.float32, name="acc",
                                              space="PSUM", tag="ac", bufs=4)
                        nc.tensor.matmul(live[t][:], lhsT=P4[:, i, :],
                                         rhs=rhs4[:, i, :],
                                         start=bool(sched.starts[c]),
                                         stop=bool(sched.stops[c]))
                        if sched.stops[c]:
                            yield t, live.pop(t)

            def pass2(ev, bias_t, resid_tile, outbuf, t_wst,
                      aso, ado, gsrc):
                # self-loop term computed locally (no gather):
                # evs = exp(leakyrelu(a_s_own + a_d_own))
                ala = wp.tile([128, 4], dt.float32, tag="ala")
                nc.vector.tensor_tensor(out=ala[:], in0=aso, in1=ado,
                                        op=mybir.AluOpType.add)
                nc.vector.scalar_tensor_tensor(
                    out=ala[:], in0=ala[:], scalar=0.2, in1=ala[:],
                    op0=mybir.AluOpType.mult, op1=mybir.AluOpType.max)
                evs = wp.tile([128, 4], dt.float32, tag="evs")
                nc.scalar.activation(evs[:], ala[:],
                                     mybir.ActivationFunctionType.Exp)
                den = wp.tile([128, 4], dt.float32, tag="den")
                nc.vector.tensor_tensor(out=den[:], in0=ev[:, 128:132],
                                        in1=evs[:], op=mybir.AluOpType.add)
                nc.vector.tensor_scalar(out=den[:], in0=den[:],
                                        scalar1=1e-16, scalar2=None,
                                        op0=mybir.AluOpType.add)
                nc.vector.reciprocal(out=den[:], in_=den[:])
                aggp = wp.tile([128, 128], dt.float32, tag="agp")
                nc.vector.tensor_tensor(
                    out=apv(aggp, 128, [(32, 4), (1, 32)]),
                    in0=apv(evs, 128, [(1, 4), (0, 32)]),
                    in1=AP(gsrc.tensor, gsrc.offset,
                           [(gsrc.ap[0][0], 128), (0, 4), (1, 32)]),
                    op=mybir.AluOpType.mult)
                nc.vector.tensor_tensor(
                    out=apv(aggp, 128, [(1, 128)]),
                    in0=apv(aggp, 128, [(1, 128)]),
                    in1=apv(ev, 128, [(1, 128)]),
                    op=mybir.AluOpType.add)
                aggn = wp.tile([128, 128], dt.bfloat16, tag="agn")
                nc.vector.tensor_tensor(
                    out=apv(aggn, 128, [(32, 4), (1, 32)]),
                    in0=apv(aggp, 128, [(32, 4), (1, 32)]),
                    in1=apv(den, 128, [(1, 4), (0, 32)]),
                    op=mybir.AluOpType.mult)
                ptq = pp.tile([128, 128], dt.bfloat16, space="PSUM",
                              tag="ptq", bufs=1)
                nc.tensor.transpose(out=ptq[:], in_=aggn[:], identity=ident[:])
                agT = wp.tile([128, 128], dt.bfloat16, tag="agT")
                nc.scalar.activation(agT[:], ptq[:],
                                     mybir.ActivationFunctionType.Copy)
                zps = pp.tile([128, 32], dt.float32, space="PSUM",
                              tag="zps", bufs=1)
                nc.tensor.matmul(zps[:], lhsT=agT[:], rhs=t_wst[:],
                                 start=True, stop=True)
                z = wp.tile([128, 32], dt.float32, tag="z")
                nc.vector.tensor_add(out=z[:], in0=zps[:], in1=bias_t[:])
                nc.vector.tensor_add(out=outbuf[:], in0=z[:], in1=resid_tile)
                return outbuf

            for tile_j, acc in edge_phase(s1, table1, t_i1, t_s1g, a_d1):
                g1t = AP(g1_own[:].tensor,
                         g1_own[:].offset + tile_j * 32,
                         [(g1_own[:].ap[0][0], 128), (1, 32)])
                gt = AP(g_own[:].tensor, g_own[:].offset + tile_j * 32,
                        [(g_own[:].ap[0][0], 128), (1, 32)])
                buf = wp.tile([128, 32], dt.float32, tag="g1b")
                pass2(acc[:], t_b1, gt, buf, t_wst1,
                      a_s1o[:, bass.ts(tile_j, 4)],
                      a_d1[:, bass.ts(tile_j, 4)], gt)
                nc.vector.tensor_copy(out=g1t, in_=buf[:])
                # a_d2 tile: transpose g1 then @ A2d
                g1b = wp.tile([128, 32], dt.bfloat16, tag="g1bf")
                nc.vector.tensor_copy(out=g1b[:], in_=buf[:])
                gtp = pp.tile([128, 128], dt.bfloat16, space="PSUM",
                              tag="ptq", bufs=1)
                nc.tensor.transpose(out=gtp[0:32, :], in_=g1b[:],
                                    identity=ident[:])
                gts = wp.tile([32, 128], dt.bfloat16, tag="gts")
                nc.vector.tensor_copy(out=gts[:], in_=gtp[0:32, :])
                pad2 = pp.tile([128, 32], dt.float32, space="PSUM",
                               tag="zps", bufs=1)
                nc.tensor.matmul(pad2[:, 0:4], lhsT=gts[:], rhs=t_a2d[:],
                                 start=True, stop=True)
                nc.vector.tensor_copy(out=a_d2[:, bass.ts(tile_j, 4)],
                                      in_=pad2[:, 0:4])

            for tile_j, acc in edge_phase(s2, table2, t_i2, t_s2g, a_d2):
                g1t = AP(g1_own[:].tensor, g1_own[:].offset + tile_j * 32,
                         [(g1_own[:].ap[0][0], 128), (1, 32)])
                buf = wp.tile([128, 32], dt.float32, tag="g2b")
                dt_src = AP(d_own[:].tensor, d_own[:].offset + tile_j * 32,
                            [(d_own[:].ap[0][0], 128), (1, 32)])
                pass2(acc[:], t_b2, g1t, buf, t_wst2,
                      a_s2o[:, bass.ts(tile_j, 4)],
                      a_d2[:, bass.ts(tile_j, 4)], dt_src)
                nc.sync.dma_start(out=outp[bass.ts(tile_j, 128), :],
                                  in_=buf[:])
            edge_psum.__exit__(None, None, None)
    nc.finalize()
    return nc


# --------------------------------------------------------------------------
# entry point
# --------------------------------------------------------------------------


def kernel(x_gene, x_drug, edge_gg, edge_dg,
           Wg, bg, gg_gamma, gg_beta, Wd, bd, dg_gamma, dg_beta,
           W1, att_s1, att_d1, bias1, W2, att_s2, att_d2, bias2):
    global LAST_RESULT
    _install_patches()
    from concourse.bass_utils import run_bass_kernel_spmd
    import os

    f32 = np.float32
    x_gene = np.asarray(x_gene, f32)
    x_drug = np.asarray(x_drug, f32)

    # edges (self loops handled locally in the epilogue)
    s1 = _prep_edges(np.asarray(edge_gg[0], np.int64),
                     np.asarray(edge_gg[1], np.int64))
    s2 = _prep_edges(np.asarray(edge_dg[0], np.int64),
                     np.asarray(edge_dg[1], np.int64))

    # weights
    W1 = np.asarray(W1, f32)
    W2 = np.asarray(W2, f32)
    eye = np.eye(32, dtype=f32)
    w1comb = np.concatenate([eye, _fold_att(W1, att_s1),
                             _fold_att(W1, att_d1)], axis=1)   # [32, 40]
    w2comb = np.concatenate([eye, _fold_att(W2, att_s2)], axis=1)  # [32, 36]
    a2d = _fold_att(W2, att_d2).astype(bf16)     # [32, 4]

    npad = RPAD - RN
    relu_bg = np.maximum(np.asarray(bg, f32), 0)
    relu_bd = np.maximum(np.asarray(bd, f32), 0)
    statc = np.stack([npad * relu_bg, npad * relu_bg**2,
                      npad * relu_bd, npad * relu_bd**2], axis=1)
    gamb = np.stack([np.asarray(gg_gamma, f32), np.asarray(gg_beta, f32),
                     np.asarray(dg_gamma, f32), np.asarray(dg_beta, f32)],
                    axis=1)

    nc = _build(s1, s2)

    in_maps = []
    for k in range(M):
        xg_s = np.zeros((RPAD, 128), f32)
        xg_s[:RN] = x_gene[k * RN:(k + 1) * RN]
        xd_s = np.zeros((RPAD, 96), f32)
        xd_s[:RN] = x_drug[k * RN:(k + 1) * RN]
        in_maps.append({
            "xg": np.ascontiguousarray(xg_s.T).astype(bf16),
            "xd": np.ascontiguousarray(xd_s.T).astype(bf16),
            "wg": np.asarray(Wg, f32).astype(bf16),
            "wd": np.asarray(Wd, f32).astype(bf16),
            "bgv": np.asarray(bg, f32).reshape(32, 1),
            "bdv": np.asarray(bd, f32).reshape(32, 1),
            "statc": statc.astype(f32),
            "gamb": gamb.astype(f32),
            "w1c": w1comb,
            "w2c": w2comb,
            "a2d": a2d,
            "wst1": _wstack(W1).astype(bf16),
            "wst2": _wstack(W2).astype(bf16),
            "bias1r": np.broadcast_to(np.asarray(bias1, f32), (128, 32)).copy(),
            "bias2r": np.broadcast_to(np.asarray(bias2, f32), (128, 32)).copy(),
            "iotap": np.broadcast_to(np.arange(128), (128, 128)).astype(bf16).copy(),
            "i1": s1.planes[k],
            "g1s": s1.segs[k],
            "i2": s2.planes[k],
            "g2s": s2.segs[k],
        })

    trace = bool(os.environ.get("TRNGNN_TRACE"))
    res = run_bass_kernel_spmd(nc, in_maps, core_ids=list(range(M)),
                               trace=trace)
    LAST_RESULT = res

    out = np.empty((NG, 32), f32)
    for k in range(M):
        out[k * RN:(k + 1) * RN] = res.results[k]["out"][:RN]
    return out



# revision 33
# speedup vs baseline: 1.0247x; 1.0247x over previous
"""Trainium2 Bass kernel for nn_Columbina_Model (2-layer hetero GAT).

Strategy: dst-shard gene nodes across 8 cores (12500 each, padded to 12544).
Host sorts each core's edges by (dst tile, src region), groups them into
128-edge chunks per (tile, region). On device: encoders + BN stats
(AllReduce) per-shard; BN folded into slim table rows
[g post-BN (32 bf16) | a_s (4 bf16) | pad] of 256B; tables AllGathered.
Edge phase gathers rows via batched SWDGE dma_gather (one instruction per
run of up to 8 chunks sharing a src region — int16 region-local indices),
computes attention (leaky-relu + exp), and segment-sums per-head
exp-weighted g into per-node-tile PSUM accumulators with one-hot matmuls.
Per-tile epilogue normalizes per head and reconstructs the H*C GAT output
with a single stacked-W matmul, then adds bias + residual.
"""
import json
import sys
import types

import numpy as np
import ml_dtypes

H, C = 4, 32
NG = ND = 100000
M = 8
RN = 12500
RPAD = 12544            # 98 * 128
NTILES = RPAD // 128
NT = RPAD * M           # 100352 table rows
DUMMY = RN              # dummy table row (core-0 own pad; a_s=-1e9)
ROWW = 36               # table row elements (bf16) = 72B
GB = 8                  # chunks per compute group
BN_EPS = 1e-5

bf16 = ml_dtypes.bfloat16

LAST_RESULT = None

# --------------------------------------------------------------------------
# runtime patches (this container lacks antenv.axon_hooks; walrus rejects >1
# sync wait per instruction)
# --------------------------------------------------------------------------


def _install_patches():
    if "antenv.axon_hooks" not in sys.modules:
        try:
            import antenv
            from trn_agent_boot.trn_boot import _ntff_profile_via_ctypes

            m = types.ModuleType("antenv.axon_hooks")
            _state = {"hook": _ntff_profile_via_ctypes("/opt/axon/libaxon_pjrt.so")}
            m.get_axon_ntff_profile_hook = lambda: _state["hook"]
            m.set_axon_ntff_profile_hook = lambda h: _state.__setitem__("hook", h)
            sys.modules["antenv.axon_hooks"] = m
            antenv.axon_hooks = m
        except Exception:
            pass

    import concourse.tile as tile
    from concourse.vector_clock import ScopedClock

    if not getattr(tile.TileContext, "_drain_patched", False):
        def _drain_and_barrier(self, tick_clock, wait_clock):
            gc = tick_clock.global_clock
            for proc in range(len(gc)):
                tick = gc[proc]
                if tick > 0:
                    partial = ScopedClock()
                    partial.require_at_least(None, proc, tick)
                    nop = self.nc.sync.nop(nofuse=True)
                    wait_clock.add_sem_waits(nop.ins, partial)
            self.nc.sync.drain()
            self.nc.all_engine_barrier()
            assert self.sems is not None
            popped = self.nc._tile_sem_poison_stack.pop()
            assert popped is self._sem_poison
            self.nc.clear_and_free_semaphores(list(self.sems.allocated().values()))
            self.nc.all_engine_barrier()

        tile.TileContext._drain_and_barrier = _drain_and_barrier
        tile.TileContext._drain_patched = True

    import concourse.bass_utils as bu
    import concourse.bass2jax as b2j

    if not getattr(bu, "_compile_patched", False):
        def _split(bir_json):
            d = json.loads(bir_json)
            for f in d.get("functions", []):
                for bb in f.get("blocks", []):
                    new_insts = []
                    for inst in bb["instructions"]:
                        si = inst.get("sync_info")
                        ow = (si or {}).get("on_wait") or []
                        if len(ow) > 1:
                            for i, w in enumerate(ow[:-1]):
                                new_insts.append({
                                    "debug": inst.get("debug", 0),
                                    "engine": inst["engine"],
                                    "ins": [], "outs": [],
                                    "name": f"{inst['name']}-w{i}",
                                    "opcode": "NoOp",
                                    "sync_info": {"on_update": [],
                                                  "on_wait": [w]},
                                })
                            si["on_wait"] = ow[-1:]
                        new_insts.append(inst)
                    bb["instructions"] = new_insts
            return json.dumps(d).encode()

        orig = bu.compile_bir_kernel

        def compile_bir_kernel(bir_json, tmpdir, neff_name="file.neff"):
            return orig(_split(bir_json), tmpdir, neff_name)

        bu.compile_bir_kernel = compile_bir_kernel
        b2j.compile_bir_kernel = compile_bir_kernel
        bu._compile_patched = True


# --------------------------------------------------------------------------
# host-side edge preprocessing
# --------------------------------------------------------------------------


def _fold_att(W, att):
    # W [32, 128], att [H, C] -> [32, H]
    return np.einsum('jhc,hc->jh', np.asarray(W).reshape(32, H, C), np.asarray(att))


def _remap(v):
    return (v // RN) * RPAD + (v % RN)


class _Sched:
    pass


def _prep_edges(src, dst):
    """Chunk schedule shared by all cores + per-core idx/seg planes."""
    src = _remap(np.asarray(src).astype(np.int64))
    dst = np.asarray(dst).astype(np.int64)

    percore = []
    counts = np.zeros((M, NTILES), np.int64)
    for k in range(M):
        m = (dst >= k * RN) & (dst < (k + 1) * RN)
        d = dst[m] - k * RN
        t = d >> 7
        order = np.lexsort((src[m], t))  # by (tile, src idx)
        percore.append((src[m][order], (d & 127)[order], t[order]))
        counts[k] = np.bincount(t, minlength=NTILES)

    cpt = np.maximum(1, (counts.max(axis=0) + 127) // 128)
    nch = int(cpt.sum())
    chunk_base = np.concatenate([[0], np.cumsum(cpt)])[:-1]
    tiles = np.repeat(np.arange(NTILES), cpt)

    starts = np.zeros(nch, bool)
    stops = np.zeros(nch, bool)
    starts[0] = True
    starts[1:] = tiles[1:] != tiles[:-1]
    stops[-1] = True
    stops[:-1] = tiles[1:] != tiles[:-1]

    idx32 = np.full((M, nch, 128), DUMMY, np.int32)
    seg = np.zeros((M, nch, 128), np.float32)
    for k in range(M):
        sk, dk, tk = percore[k]
        tile_start = np.concatenate([[0], np.cumsum(counts[k])])[:-1]
        pos = np.arange(len(sk)) - tile_start[tk]
        gchunk = chunk_base[tk] + (pos >> 7)
        slot = pos & 127
        idx32[k, gchunk, slot] = sk
        seg[k, gchunk, slot] = dk

    s = _Sched()
    s.nch = nch
    s.tiles, s.starts, s.stops = tiles, starts, stops
    s.planes = np.ascontiguousarray(np.transpose(idx32, (0, 2, 1)))
    s.segs = np.ascontiguousarray(np.transpose(seg, (0, 2, 1))).astype(bf16)
    return s


def _wstack(W):
    # W [32, H*C] -> [H*C, 32]: out[h*32+j, c] = W[j, h*32+c] / H
    Wr = np.asarray(W, np.float32).reshape(32, H, C)
    return (np.transpose(Wr, (1, 0, 2)).reshape(H * C, C) / H)


# --------------------------------------------------------------------------
# device kernel builder
# --------------------------------------------------------------------------


def _build(s1, s2):
    import concourse.bass as bass
    import concourse.mybir as mybir
    import concourse.tile as tile
    from concourse import bacc
    from concourse.bass import AP, IndirectOffsetOnAxis

    dt = mybir.dt
    nc = bacc.Bacc()

    def par(name, shape, dty, out=False):
        return nc.declare_dram_parameter(name, shape, dty, isOutput=out)

    xg = par("xg", [128, RPAD], dt.bfloat16)        # transposed gene shard
    xd = par("xd", [96, RPAD], dt.bfloat16)         # transposed drug shard
    wg = par("wg", [128, 32], dt.bfloat16)
    wd = par("wd", [96, 32], dt.bfloat16)
    bgv = par("bgv", [32, 1], dt.float32)           # encoder biases
    bdv = par("bdv", [32, 1], dt.float32)
    statc = par("statc", [32, 4], dt.float32)       # pad-col stat corrections
    gamb = par("gamb", [32, 4], dt.float32)         # gamma/beta g, d
    w1c = par("w1c", [32, 40], dt.float32)          # [I | A1s | A1d]
    w2c = par("w2c", [32, 36], dt.float32)          # [I | A2s]
    a2d = par("a2d", [32, 4], dt.bfloat16)          # fold(W2, att_d2)
    wst1 = par("wst1", [128, 32], dt.bfloat16)      # stacked W1 / H
    wst2 = par("wst2", [128, 32], dt.bfloat16)      # stacked W2 / H
    bias1r = par("bias1r", [128, 32], dt.float32)
    bias2r = par("bias2r", [128, 32], dt.float32)
    iotap = par("iotap", [128, 128], dt.bfloat16)
    i1 = par("i1", [128, s1.nch], dt.int32)
    g1s = par("g1s", [128, s1.nch], dt.bfloat16)
    i2 = par("i2", [128, s2.nch], dt.int32)
    g2s = par("g2s", [128, s2.nch], dt.bfloat16)
    outp = par("out", [RPAD, 32], dt.float32, out=True)

    with tile.TileContext(nc) as tc:
        with (
            tc.tile_pool(name="const", bufs=1) as cp,
            tc.tile_pool(name="enc", bufs=2) as ep,
            tc.tile_pool(name="work", bufs=4) as wp,
            tc.tile_pool(name="dram", bufs=1, space="DRAM") as dp,
        ):
            from concourse.masks import make_identity
            # ---------------- constants / params to SBUF ----------------
            t_wg = cp.tile([128, 32], dt.bfloat16)
            t_wd = cp.tile([96, 32], dt.bfloat16)
            t_bg = cp.tile([32, 1], dt.float32)
            t_bd = cp.tile([32, 1], dt.float32)
            t_statc = cp.tile([32, 4], dt.float32)
            t_gamb = cp.tile([32, 4], dt.float32)
            t_w1c = cp.tile([32, 40], dt.float32)
            t_w2c = cp.tile([32, 36], dt.float32)
            t_a2d = cp.tile([32, 4], dt.bfloat16)
            t_wst1 = cp.tile([128, 32], dt.bfloat16)
            t_wst2 = cp.tile([128, 32], dt.bfloat16)
            t_b1 = cp.tile([128, 32], dt.float32)
            t_b2 = cp.tile([128, 32], dt.float32)
            t_iota = cp.tile([128, 128], dt.bfloat16)
            t_i1 = cp.tile([128, s1.nch], dt.int32)
            t_s1g = cp.tile([128, s1.nch], dt.bfloat16)
            t_i2 = cp.tile([128, s2.nch], dt.int32)
            t_s2g = cp.tile([128, s2.nch], dt.bfloat16)
            ident = cp.tile([128, 128], dt.bfloat16)
            make_identity(nc, ident[:])
            for t, p in [(t_wg, wg), (t_wd, wd), (t_bg, bgv), (t_bd, bdv),
                         (t_statc, statc), (t_gamb, gamb), (t_w1c, w1c),
                         (t_w2c, w2c), (t_a2d, a2d), (t_wst1, wst1),
                         (t_wst2, wst2), (t_b1, bias1r), (t_b2, bias2r),
                         (t_iota, iotap), (t_i1, i1), (t_s1g, g1s),
                         (t_i2, i2), (t_s2g, g2s)]:
                nc.sync.dma_start(out=t[:], in_=p[:])


            # DRAM internals
            table1 = dp.tile([NT, ROWW], dt.bfloat16)
            table2 = dp.tile([NT, ROWW], dt.bfloat16)
            own1 = dp.tile([RPAD, ROWW], dt.bfloat16)
            own2 = dp.tile([RPAD, ROWW], dt.bfloat16)
            st_in = dp.tile([32, 4], dt.float32)
            st_out = dp.tile([32, 4], dt.float32)

            yp = tc.tile_pool(name="yp", bufs=1)
            ypool = yp.__enter__()
            enc_psum = tc.tile_pool(name="ps_enc", bufs=2, space="PSUM")
            pp = enc_psum.__enter__()

            # ---------------- encoders: y_T = relu(W^T x_T + b) ---------
            def encode(xT, wT, bT, npart, tag):
                yT = ypool.tile([33, RPAD], dt.bfloat16, tag=f"y{tag}")
                st = ep.tile([32, 50], dt.float32, tag=f"st{tag}")
                for i in range(25):
                    lo = i * 512
                    w = min(512, RPAD - lo)
                    x512 = ep.tile([128, 512], dt.bfloat16, tag=f"x{tag}",
                                   bufs=3)
                    nc.sync.dma_start(out=x512[:npart, :w],
                                      in_=xT[:, lo:lo + w])
                    ps = pp.tile([32, 512], dt.float32, space="PSUM", tag="encp")
                    nc.tensor.matmul(ps[:, :w], lhsT=wT[:npart, :],
                                     rhs=x512[:npart, :w],
                                     start=True, stop=True)
                    nc.scalar.activation(
                        yT[0:32, lo:lo + w], ps[:, :w],
                        mybir.ActivationFunctionType.Relu, bias=bT[:],
                        accum_out=st[:, i:i + 1])
                    sq = ep.tile([32, 512], dt.float32, tag=f"sq{tag}")
                    nc.scalar.activation(
                        sq[:, :w], yT[0:32, lo:lo + w],
                        mybir.ActivationFunctionType.Square,
                        accum_out=st[:, 25 + i:26 + i])
                nc.vector.memset(yT[32:33, :], 1.0)
                s1t = ep.tile([32, 2], dt.float32, tag=f"s1{tag}")
                nc.vector.tensor_reduce(s1t[:, 0:1], st[:, 0:25],
                                        mybir.AxisListType.X, mybir.AluOpType.add)
                nc.vector.tensor_reduce(s1t[:, 1:2], st[:, 25:50],
                                        mybir.AxisListType.X, mybir.AluOpType.add)
                return yT, s1t

            ygT, sg = encode(xg, t_wg, t_bg, 128, "g")
            ydT, sd = encode(xd, t_wd, t_bd, 96, "d")

            # stats allreduce: [32,4] = [sum_g, sq_g, sum_d, sq_d] - corr
            packed = ep.tile([32, 4], dt.float32, tag="pk")
            nc.vector.tensor_copy(out=packed[:, 0:2], in_=sg[:])
            nc.vector.tensor_copy(out=packed[:, 2:4], in_=sd[:])
            nc.vector.tensor_sub(out=packed[:], in0=packed[:], in1=t_statc[:])
            nc.sync.dma_start(out=st_in[:], in_=packed[:])
            nc.gpsimd.collective_compute(
                "AllReduce", mybir.AluOpType.add,
                replica_groups=[list(range(M))],
                ins=[st_in.opt()], outs=[st_out.opt()])
            tstat = ep.tile([32, 4], dt.float32, tag="ts")
            nc.gpsimd.dma_start(out=tstat[:], in_=st_out[:])

            # BN fold: s = gamma/sqrt(var+eps), t = beta - mu*s  (cols: g, d)
            sfold = ep.tile([32, 2], dt.float32, tag="sf")
            tfold = ep.tile([32, 2], dt.float32, tag="tf")
            mu = ep.tile([32, 2], dt.float32, tag="mu")
            var = ep.tile([32, 2], dt.float32, tag="va")
            tmp = ep.tile([32, 2], dt.float32, tag="tm")
            nc.vector.tensor_scalar(out=mu[:, 0:1], in0=tstat[:, 0:1],
                                    scalar1=1.0 / NG, scalar2=None,
                                    op0=mybir.AluOpType.mult)
            nc.vector.tensor_scalar(out=mu[:, 1:2], in0=tstat[:, 2:3],
                                    scalar1=1.0 / ND, scalar2=None,
                                    op0=mybir.AluOpType.mult)
            nc.vector.tensor_scalar(out=var[:, 0:1], in0=tstat[:, 1:2],
                                    scalar1=1.0 / NG, scalar2=None,
                                    op0=mybir.AluOpType.mult)
            nc.vector.tensor_scalar(out=var[:, 1:2], in0=tstat[:, 3:4],
                                    scalar1=1.0 / ND, scalar2=None,
                                    op0=mybir.AluOpType.mult)
            nc.vector.tensor_mul(out=tmp[:], in0=mu[:], in1=mu[:])
            nc.vector.tensor_sub(out=var[:], in0=var[:], in1=tmp[:])
            nc.vector.tensor_scalar(out=var[:], in0=var[:], scalar1=BN_EPS,
                                    scalar2=None, op0=mybir.AluOpType.add)
            nc.scalar.activation(tmp[:], var[:], mybir.ActivationFunctionType.Sqrt)
            nc.vector.reciprocal(out=tmp[:], in_=tmp[:])
            nc.vector.tensor_mul(out=sfold[:, 0:1], in0=t_gamb[:, 0:1],
                                 in1=tmp[:, 0:1])
            nc.vector.tensor_mul(out=sfold[:, 1:2], in0=t_gamb[:, 2:3],
                                 in1=tmp[:, 1:2])
            nc.vector.tensor_mul(out=tfold[:], in0=mu[:], in1=sfold[:])
            nc.vector.tensor_sub(out=tfold[:, 0:1], in0=t_gamb[:, 1:2],
                                 in1=tfold[:, 0:1])
            nc.vector.tensor_sub(out=tfold[:, 1:2], in0=t_gamb[:, 3:4],
                                 in1=tfold[:, 1:2])

            # waug [33, n]: rows 0..32 = diag-scaled [I|A..], row 32 = t@[..]
            def build_waug(wcomb, ncols, sf_col, tf_col, tag):
                wa = cp.tile([33, ncols], dt.bfloat16, tag=f"wa{tag}")
                scaled = ep.tile([32, ncols], dt.float32, tag=f"sc{tag}")
                nc.vector.tensor_tensor(
                    out=scaled[:], in0=wcomb[:, :ncols],
                    in1=sf_col.to_broadcast([32, ncols]),
                    op=mybir.AluOpType.mult)
                nc.vector.tensor_copy(out=wa[0:32, :], in_=scaled[:])
                tr = pp.tile([1, 40], dt.float32, space="PSUM", tag="warow")
                tscaled = ep.tile([32, 1], dt.float32, tag=f"tsc{tag}")
                nc.vector.tensor_copy(out=tscaled[:], in_=tf_col)
                nc.tensor.matmul(tr[:, :ncols], lhsT=tscaled[:],
                                 rhs=wcomb[:, :ncols], start=True, stop=True)
                nc.vector.tensor_copy(out=wa[32:33, :], in_=tr[:, :ncols])
                return wa

            waug1 = build_waug(t_w1c, 40, sfold[:, 0:1], tfold[:, 0:1], "1")
            waug2 = build_waug(t_w2c, 36, sfold[:, 1:2], tfold[:, 1:2], "2")

            # ---------------- own table builds ----------------
            a_d1 = cp.tile([128, NTILES * 4], dt.bfloat16)
            a_d2 = cp.tile([128, NTILES * 4], dt.bfloat16)
            a_s1o = cp.tile([128, NTILES * 4], dt.bfloat16)
            a_s2o = cp.tile([128, NTILES * 4], dt.bfloat16)
            g_own = cp.tile([128, NTILES * 32], dt.float32)
            d_own = cp.tile([128, NTILES * 32], dt.float32)
            g1_own = cp.tile([128, NTILES * 32], dt.float32)

            for j in range(NTILES):
                ps = pp.tile([128, 40], dt.float32, space="PSUM", tag="tb")
                nc.tensor.matmul(ps[:], lhsT=ygT[:, bass.ts(j, 128)],
                                 rhs=waug1[:], start=True, stop=True)
                row = wp.tile([128, ROWW], dt.bfloat16, tag="tbr")
                nc.vector.tensor_copy(out=row[:, 0:36], in_=ps[:, 0:36])
                nc.sync.dma_start(out=own1[bass.ts(j, 128), :], in_=row[:])
                nc.vector.tensor_copy(out=a_d1[:, bass.ts(j, 4)],
                                      in_=ps[:, 36:40])
                nc.vector.tensor_copy(out=a_s1o[:, bass.ts(j, 4)],
                                      in_=ps[:, 32:36])
                nc.vector.tensor_copy(out=g_own[:, bass.ts(j, 32)],
                                      in_=ps[:, 0:32])
            for j in range(NTILES):
                ps = pp.tile([128, 36], dt.float32, space="PSUM", tag="tb2")
                nc.tensor.matmul(ps[:], lhsT=ydT[:, bass.ts(j, 128)],
                                 rhs=waug2[:], start=True, stop=True)
                row = wp.tile([128, ROWW], dt.bfloat16, tag="tbr2")
                nc.vector.tensor_copy(out=row[:, 0:36], in_=ps[:])
                nc.sync.dma_start(out=own2[bass.ts(j, 128), :], in_=row[:])
                nc.vector.tensor_copy(out=a_s2o[:, bass.ts(j, 4)],
                                      in_=ps[:, 32:36])
                nc.vector.tensor_copy(out=d_own[:, bass.ts(j, 32)],
                                      in_=ps[:, 0:32])

            # dummy rows in the own-shard pad region (a_s = -1e9 -> exp = 0)
            drow = wp.tile([8, ROWW], dt.bfloat16, tag="drow")
            nc.vector.memset(drow[:], 0.0)
            nc.vector.memset(drow[:, 32:36], -1e9)
            nc.sync.dma_start(out=own1[RN:RN + 8, :], in_=drow[:])
            nc.sync.dma_start(out=own2[RN:RN + 8, :], in_=drow[:])

            yp.__exit__(None, None, None)
            enc_psum.__exit__(None, None, None)
            edge_psum = tc.tile_pool(name="ps_edge", bufs=2, space="PSUM")
            pp = edge_psum.__enter__()
            # AllGather tables
            nc.gpsimd.collective_compute(
                "AllGather", mybir.AluOpType.bypass,
                replica_groups=[list(range(M))],
                ins=[own1.opt()], outs=[table1[:]])
            nc.gpsimd.collective_compute(
                "AllGather", mybir.AluOpType.bypass,
                replica_groups=[list(range(M))],
                ins=[own2.opt()], outs=[table2[:]])

            # ---------------- edge phase ----------------
            def apv(t, part, dims, offset=0):
                a = t[:]
                return AP(a.tensor, a.offset + offset,
                          [(a.ap[0][0], part)] + list(dims))

            def edge_phase(sched, table, t_idx, t_sg, a_d):
                """Batched edge pipeline; yields (tile, psum_acc) per dst
                node tile."""
                live = {}
                for c0 in range(0, sched.nch, GB):
                    G = min(GB, sched.nch - c0)
                    g4 = wp.tile([128, GB, ROWW], dt.bfloat16, tag="eg",
                                 bufs=8)
                    for i in range(G):
                        nc.gpsimd.indirect_dma_start(
                            out=g4[:, i, :], out_offset=None, in_=table[:],
                            in_offset=IndirectOffsetOnAxis(
                                ap=t_idx[:, c0 + i:c0 + i + 1], axis=0))
                    P4 = wp.tile([128, GB, 128], dt.bfloat16, tag="eP", bufs=3)
                    pst = t_sg[:].ap[0][0]
                    nc.vector.tensor_tensor(
                        out=P4[:, :G, :],
                        in0=AP(t_sg[:].tensor, t_sg[:].offset + c0,
                               [(pst, 128), (1, G), (0, 128)]),
                        in1=apv(t_iota, 128, [(0, G), (1, 128)]),
                        op=mybir.AluOpType.is_equal)
                    ptp4 = pp.tile([128, GB, 128], dt.bfloat16, space="PSUM",
                                   tag="ptp", bufs=1)
                    for i in range(G):
                        nc.tensor.transpose(out=ptp4[:, i, :], in_=P4[:, i, :],
                                            identity=ident[:])
                    PT4 = wp.tile([128, GB, 128], dt.bfloat16, tag="ePT",
                                  bufs=3)
                    nc.scalar.activation(
                        apv(PT4, 128, [(1, G * 128)]),
                        apv(ptp4, 128, [(1, G * 128)]),
                        mybir.ActivationFunctionType.Copy)
                    pa4 = pp.tile([128, GB, 4], dt.float32, space="PSUM",
                                  tag="pa", bufs=1)
                    for i in range(G):
                        nc.tensor.matmul(
                            pa4[:, i, :], lhsT=PT4[:, i, :],
                            rhs=a_d[:, bass.ts(int(sched.tiles[c0 + i]), 4)],
                            start=True, stop=True)
                    alpha4 = wp.tile([128, GB, 4], dt.float32, tag="eal",
                                     bufs=3)
                    nc.vector.tensor_tensor(
                        out=alpha4[:, :G, :],
                        in0=apv(g4, 128, [(ROWW, G), (1, 4)], offset=32),
                        in1=apv(pa4, 128, [(4, G), (1, 4)]),
                        op=mybir.AluOpType.add)
                    lr4 = wp.tile([128, GB, 4], dt.float32, tag="elr", bufs=3)
                    nc.vector.scalar_tensor_tensor(
                        out=apv(lr4, 128, [(1, G * 4)]),
                        in0=apv(alpha4, 128, [(1, G * 4)]), scalar=0.2,
                        in1=apv(alpha4, 128, [(1, G * 4)]),
                        op0=mybir.AluOpType.mult, op1=mybir.AluOpType.max)
                    rhs4 = wp.tile([128, GB, 132], dt.bfloat16, tag="erh",
                                   bufs=3)
                    nc.scalar.activation(
                        apv(rhs4, 128, [(132, G), (1, 4)], offset=128),
                        apv(lr4, 128, [(4, G), (1, 4)]),
                        mybir.ActivationFunctionType.Exp)
                    nc.vector.tensor_tensor(
                        out=apv(rhs4, 128, [(132, G), (32, 4), (1, 32)]),
                        in0=apv(g4, 128, [(ROWW, G), (0, 4), (1, 32)]),
                        in1=apv(rhs4, 128, [(132, G), (1, 4), (0, 32)],
                                offset=128),
                        op=mybir.AluOpType.mult)
                    for i in range(G):
                        c = c0 + i
                        t = int(sched.tiles[c])
                        if sched.starts[c]:
                            live[t] = pp.tile([128, 132], dt.float32, name="acc",
                                              space="PSUM", tag="ac", bufs=4)
                        nc.tensor.matmul(live[t][:], lhsT=P4[:, i, :],
                                         rhs=rhs4[:, i, :],
                                         start=bool(sched.starts[c]),
                                         stop=bool(sched.stops[c]))
                        if sched.stops[c]:
                            yield t, live.pop(t)

            def pass2(ev, bias_t, resid_tile, outbuf, t_wst,
                      aso, ado, gsrc):
                # self-loop term computed locally (no gather):
                # evs = exp(leakyrelu(a_s_own + a_d_own))
                ala = wp.tile([128, 4], dt.float32, tag="ala")
                nc.vector.tensor_tensor(out=ala[:], in0=aso, in1=ado,
                                        op=mybir.AluOpType.add)
                nc.vector.scalar_tensor_tensor(
                    out=ala[:], in0=ala[:], scalar=0.2, in1=ala[:],
                    op0=mybir.AluOpType.mult, op1=mybir.AluOpType.max)
                evs = wp.tile([128, 4], dt.float32, tag="evs")
                nc.scalar.activation(evs[:], ala[:],
                                     mybir.ActivationFunctionType.Exp)
                den = wp.tile([128, 4], dt.float32, tag="den")
                nc.vector.tensor_tensor(out=den[:], in0=ev[:, 128:132],
                                        in1=evs[:], op=mybir.AluOpType.add)
                nc.vector.tensor_scalar(out=den[:], in0=den[:],
                                        scalar1=1e-16, scalar2=None,
                                        op0=mybir.AluOpType.add)
                nc.vector.reciprocal(out=den[:], in_=den[:])
                aggp = wp.tile([128, 128], dt.float32, tag="agp")
                nc.vector.tensor_tensor(
                    out=apv(aggp, 128, [(32, 4), (1, 32)]),
                    in0=apv(evs, 128, [(1, 4), (0, 32)]),
                    in1=AP(gsrc.tensor, gsrc.offset,
                           [(gsrc.ap[0][0], 128), (0, 4), (1, 32)]),
                    op=mybir.AluOpType.mult)
                nc.vector.tensor_tensor(
                    out=apv(aggp, 128, [(1, 128)]),
                    in0=apv(aggp, 128, [(1, 128)]),
                    in1=apv(ev, 128, [(1, 128)]),
                    op=mybir.AluOpType.add)
                aggn = wp.tile([128, 128], dt.bfloat16, tag="agn")
                nc.vector.tensor_tensor(
                    out=apv(aggn, 128, [(32, 4), (1, 32)]),
                    in0=apv(aggp, 128, [(32, 4), (1, 32)]),
                    in1=apv(den, 128, [(1, 4), (0, 32)]),
                    op=mybir.AluOpType.mult)
                ptq = pp.tile([128, 128], dt.bfloat16, space="PSUM",
                              tag="ptq", bufs=1)
                nc.tensor.transpose(out=ptq[:], in_=aggn[:], identity=ident[:])
                agT = wp.tile([128, 128], dt.bfloat16, tag="agT")
                nc.scalar.activation(agT[:], ptq[:],
                                     mybir.ActivationFunctionType.Copy)
                zps = pp.tile([128, 32], dt.float32, space="PSUM",
                              tag="zps", bufs=1)
                nc.tensor.matmul(zps[:], lhsT=agT[:], rhs=t_wst[:],
                                 start=True, stop=True)
                z = wp.tile([128, 32], dt.float32, tag="z")
                nc.vector.tensor_add(out=z[:], in0=zps[:], in1=bias_t[:])
                nc.vector.tensor_add(out=outbuf[:], in0=z[:], in1=resid_tile)
                return outbuf

            for tile_j, acc in edge_phase(s1, table1, t_i1, t_s1g, a_d1):
                g1t = AP(g1_own[:].tensor,
                         g1_own[:].offset + tile_j * 32,
                         [(g1_own[:].ap[0][0], 128), (1, 32)])
                gt = AP(g_own[:].tensor, g_own[:].offset + tile_j * 32,
                        [(g_own[:].ap[0][0], 128), (1, 32)])
                buf = wp.tile([128, 32], dt.float32, tag="g1b")
                pass2(acc[:], t_b1, gt, buf, t_wst1,
                      a_s1o[:, bass.ts(tile_j, 4)],
                      a_d1[:, bass.ts(tile_j, 4)], gt)
                nc.vector.tensor_copy(out=g1t, in_=buf[:])
                # a_d2 tile: transpose g1 then @ A2d
                g1b = wp.tile([128, 32], dt.bfloat16, tag="g1bf")
                nc.vector.tensor_copy(out=g1b[:], in_=buf[:])
                gtp = pp.tile([128, 128], dt.bfloat16, space="PSUM",
                              tag="ptq", bufs=1)
                nc.tensor.transpose(out=gtp[0:32, :], in_=g1b[:],
                                    identity=ident[:])
                gts = wp.tile([32, 128], dt.bfloat16, tag="gts")
                nc.vector.tensor_copy(out=gts[:], in_=gtp[0:32, :])
                pad2 = pp.tile([128, 32], dt.float32, space="PSUM",
                               tag="zps", bufs=1)
                nc.tensor.matmul(pad2[:, 0:4], lhsT=gts[:], rhs=t_a2d[:],
                                 start=True, stop=True)
                nc.vector.tensor_copy(out=a_d2[:, bass.ts(tile_j, 4)],
                                      in_=pad2[:, 0:4])

            for tile_j, acc in edge_phase(s2, table2, t_i2, t_s2g, a_d2):
                g1t = AP(g1_own[:].tensor, g1_own[:].offset + tile_j * 32,
                         [(g1_own[:].ap[0][0], 128), (1, 32)])
                buf = wp.tile([128, 32], dt.float32, tag="g2b")
                dt_src = AP(d_own[:].tensor, d_own[:].offset + tile_j * 32,
                            [(d_own[:].ap[0][0], 128), (1, 32)])
                pass2(acc[:], t_b2, g1t, buf, t_wst2,
                      a_s2o[:, bass.ts(tile_j, 4)],
                      a_d2[:, bass.ts(tile_j, 4)], dt_src)
                nc.sync.dma_start(out=outp[bass.ts(tile_j, 128), :],
                                  in_=buf[:])
            edge_psum.__exit__(None, None, None)
    nc.finalize()
    return nc


# --------------------------------------------------------------------------
# entry point
# --------------------------------------------------------------------------


def kernel(x_gene, x_drug, edge_gg, edge_dg,
           Wg, bg, gg_gamma, gg_beta, Wd, bd, dg_gamma, dg_beta,
           W1, att_s1, att_d1, bias1, W2, att_s2, att_d2, bias2):
    global LAST_RESULT
    _install_patches()
    from concourse.bass_utils import run_bass_kernel_spmd
    import os

    f32 = np.float32
    x_gene = np.asarray(x_gene, f32)
    x_drug = np.asarray(x_drug, f32)

    # edges (self loops handled locally in the epilogue)
    s1 = _prep_edges(np.asarray(edge_gg[0], np.int64),
                     np.asarray(edge_gg[1], np.int64))
    s2 = _prep_edges(np.asarray(edge_dg[0], np.int64),
                     np.asarray(edge_dg[1], np.int64))

    # weights
    W1 = np.asarray(W1, f32)
    W2 = np.asarray(W2, f32)
    eye = np.eye(32, dtype=f32)
    w1comb = np.concatenate([eye, _fold_att(W1, att_s1),
                             _fold_att(W1, att_d1)], axis=1)   # [32, 40]
    w2comb = np.concatenate([eye, _fold_att(W2, att_s2)], axis=1)  # [32, 36]
    a2d = _fold_att(W2, att_d2).astype(bf16)     # [32, 4]

    npad = RPAD - RN
    relu_bg = np.maximum(np.asarray(bg, f32), 0)
    relu_bd = np.maximum(np.asarray(bd, f32), 0)
    statc = np.stack([npad * relu_bg, npad * relu_bg**2,
                      npad * relu_bd, npad * relu_bd**2], axis=1)
    gamb = np.stack([np.asarray(gg_gamma, f32), np.asarray(gg_beta, f32),
                     np.asarray(dg_gamma, f32), np.asarray(dg_beta, f32)],
                    axis=1)

    nc = _build(s1, s2)

    in_maps = []
    for k in range(M):
        xg_s = np.zeros((RPAD, 128), f32)
        xg_s[:RN] = x_gene[k * RN:(k + 1) * RN]
        xd_s = np.zeros((RPAD, 96), f32)
        xd_s[:RN] = x_drug[k * RN:(k + 1) * RN]
        in_maps.append({
            "xg": np.ascontiguousarray(xg_s.T).astype(bf16),
            "xd": np.ascontiguousarray(xd_s.T).astype(bf16),
            "wg": np.asarray(Wg, f32).astype(bf16),
            "wd": np.asarray(Wd, f32).astype(bf16),
            "bgv": np.asarray(bg, f32).reshape(32, 1),
            "bdv": np.asarray(bd, f32).reshape(32, 1),
            "statc": statc.astype(f32),
            "gamb": gamb.astype(f32),
            "w1c": w1comb,
            "w2c": w2comb,
            "a2d": a2d,
            "wst1": _wstack(W1).astype(bf16),
            "wst2": _wstack(W2).astype(bf16),
            "bias1r": np.broadcast_to(np.asarray(bias1, f32), (128, 32)).copy(),
            "bias2r": np.broadcast_to(np.asarray(bias2, f32), (128, 32)).copy(),
            "iotap": np.broadcast_to(np.arange(128), (128, 128)).astype(bf16).copy(),
            "i1": s1.planes[k],
            "g1s": s1.segs[k],
            "i2": s2.planes[k],
            "g2s": s2.segs[k],
        })

    trace = bool(os.environ.get("TRNGNN_TRACE"))
    res = run_bass_kernel_spmd(nc, in_maps, core_ids=list(range(M)),
                               trace=trace)
    LAST_RESULT = res

    out = np.empty((NG, 32), f32)
    for k in range(M):
        out[k * RN:(k + 1) * RN] = res.results[k]["out"][:RN]
    return out

